# revision 5
# baseline (speedup 1.0000x reference)
"""Trainium2 Bass kernel v4 for nn_DocREModel_Triangle — collective-free,
phase-interleaved.

v4 over v3 (trace-driven):
  - full per-chunk pipeline: for each 288-pair chunk, products (DVE) ->
    normalization -> rsT -> extractors -> phase-2 feature+classifier are
    emitted together, so the PE works on chunk k's matmuls while the DVE
    computes chunk k+1's pair products (v3 ran the ~90us DVE product
    phase with the PE mostly idle, then a PE-bound phase 2).
  - gpsimd no longer issues DMAs (software-DGE dispatch was ~620ns per
    dma_start, 60us total) and no longer runs the big product muls
    (7.1us each vs 1.9us on DVE). It only takes 1/3 of the small
    phase-2 bl muls.
  - phase-2 chunk = 288 pairs (aligned with phase 1); b1 broadcast rows
    loaded just-in-time per tile (2 stride-0 DMAs, bufs=16, issued
    alternately on the two hardware DGE queues).
  - ts stored per-chunk (4 tiles) to avoid false tile-level WAR hazards
    between chunk k's phase-2 reads and chunk k+1's extractor writes.
  - wp loaded as 4 quarter-tiles so early phase-2 matmuls of chunk 0
    don't wait for the full 9.4 MB weight load.
  - PSUM: 6 feature accumulators + one shared [128,288] f32 tag (rp/ep/
    sums/replicate/classifier) with 2 bufs = 8 banks exactly.
"""

import numpy as np
import ml_dtypes

bf16 = ml_dtypes.bfloat16

B, L, H, NH = 2, 512, 768, 12
NE, NM = 24, 4
E, BS, C = 768, 64, 97
K = E // BS
NCORE = 8
SL = 64 // NCORE                 # 8 s-values per core
NP = B * NE * NE                 # 1152 pairs
PPB = NE * NE                    # 576 pairs per batch
CW = 288                         # pair chunk (12 i x 24 j)
NCH = NP // CW                   # 4
KST = K * SL * BS                # 6144 contraction rows per core
NT2 = KST // 128                 # 48 tiles, kt2 = s*6 + a (a = k//2)


def _host_prep(inputs):
    seq = np.ascontiguousarray(inputs["sequence_output"], dtype=np.float32)
    att = np.ascontiguousarray(inputs["attention"], dtype=np.float32)
    Wh = np.asarray(inputs["Wh"], dtype=np.float32)
    Wt = np.asarray(inputs["Wt"], dtype=np.float32)
    Wp = np.asarray(inputs["Wp"], dtype=np.float32)
    Wc = np.asarray(inputs["Wc"], dtype=np.float32)
    bh = np.asarray(inputs["bh"], dtype=np.float32)
    bt = np.asarray(inputs["bt"], dtype=np.float32)
    mpos = np.asarray(inputs["mention_pos"]).astype(np.int64)

    seq_bf = np.ascontiguousarray(seq.astype(bf16))
    wt1 = np.ascontiguousarray(Wt[:H].astype(bf16))
    wt2 = np.ascontiguousarray(Wt[H:].astype(bf16))
    bt_t = np.ascontiguousarray(bt.reshape(6, 128).T.astype(np.float32))
    wc_bf = np.ascontiguousarray(Wc.astype(bf16))
    wp4 = Wp.reshape(K, 64, BS, H)

    # mention rows (m, e): row = m*24 + e  -> [96, 2, 768] bf16
    ment = np.ascontiguousarray(
        np.stack([seq[b][mpos[b].T.reshape(-1)] for b in range(B)], 1)
        .astype(bf16))
    # attention gather, transposed, m-major: [b, l, (m e h)] bf16
    attT = np.zeros((B, L, NM * NE * NH), bf16)
    for b in range(B):
        G = att[b][:, mpos[b].reshape(-1), :]                 # [12, 96, 512]
        X = G.reshape(NH, NE, NM, L).transpose(3, 2, 1, 0)    # [512, 4m, 24e, 12h]
        attT[b] = X.reshape(L, -1).astype(bf16)
    attT = np.ascontiguousarray(attT)

    in_maps = []
    for c in range(NCORE):
        PERM_c = np.array([k * 64 + SL * c + sl for k in range(K) for sl in range(SL)])
        wh1p = np.ascontiguousarray(Wh[:H][:, PERM_c].astype(bf16))   # [768, 96]
        wh2p = np.ascontiguousarray(Wh[H:][:, PERM_c].astype(bf16))
        bh_c = np.ascontiguousarray(bh[PERM_c].reshape(96, 1).astype(np.float32))
        wp_c = np.ascontiguousarray(
            wp4[:, SL * c: SL * (c + 1)].transpose(1, 0, 2, 3)
            .reshape(KST, H).astype(bf16))
        in_maps.append({
            "seq_bf": seq_bf,
            "att_T": attT,
            "ment": ment,
            "wh1p": wh1p,
            "wh2p": wh2p,
            "wt1": wt1,
            "wt2": wt2,
            "bh_c": bh_c,
            "bt_t": bt_t,
            "wp_sl": wp_c,
            "wc_bf": wc_bf,
        })
    return in_maps


def _build_consts():
    S2 = np.zeros((96, NE), dtype=bf16)
    for m in range(NM):
        for e in range(NE):
            S2[m * NE + e, e] = 1.0
    ones_bf = np.ones((128, 1), dtype=bf16)
    ones_row = np.ones((1, 128), dtype=np.float32)
    Jind = np.zeros((NE, CW), dtype=bf16)
    for il in range(12):
        for j in range(NE):
            Jind[j, il * NE + j] = 1.0
    Iind = np.zeros((NE, 2 * CW), dtype=bf16)
    for ih in range(2):
        for il in range(12):
            for j in range(NE):
                Iind[ih * 12 + il, ih * CW + il * NE + j] = 1.0
    return S2, ones_bf, ones_row, Jind, Iind


def build_bass():
    import concourse.bass as bass
    import concourse.mybir as mybir
    import concourse.tile as tile
    from concourse.bacc import Bacc

    f32 = mybir.dt.float32
    bft = mybir.dt.bfloat16
    AF = mybir.ActivationFunctionType
    ALU = mybir.AluOpType
    X = mybir.AxisListType.X

    nc = Bacc("TRN2", num_devices=NCORE)

    seq_bf = nc.dram_tensor("seq_bf", [B, L, H], bft, kind="ExternalInput")
    att_T = nc.dram_tensor("att_T", [B, L, NM * NE * NH], bft, kind="ExternalInput")
    ment = nc.dram_tensor("ment", [96, B, H], bft, kind="ExternalInput")
    wh1p = nc.dram_tensor("wh1p", [H, 96], bft, kind="ExternalInput")
    wh2p = nc.dram_tensor("wh2p", [H, 96], bft, kind="ExternalInput")
    wt1 = nc.dram_tensor("wt1", [H, E], bft, kind="ExternalInput")
    wt2 = nc.dram_tensor("wt2", [H, E], bft, kind="ExternalInput")
    bh_c = nc.dram_tensor("bh_c", [96, 1], f32, kind="ExternalInput")
    bt_t = nc.dram_tensor("bt_t", [128, 6], f32, kind="ExternalInput")
    wp_sl = nc.dram_tensor("wp_sl", [KST, H], bft, kind="ExternalInput")
    wc_bf = nc.dram_tensor("wc_bf", [H, C], bft, kind="ExternalInput")
    out_lgT = nc.dram_tensor("out_lgT", [C, NP], f32, kind="ExternalOutput")

    hs_dr = nc.dram_tensor("hs_dr", [96, NP], bft)

    S2_np, ones_np, onesrow_np, J_np, I_np = _build_consts()
    S2_dr = nc.inline_tensor(S2_np, "s2_const")
    ones_dr = nc.inline_tensor(ones_np, "ones_const")
    onesrow_dr = nc.inline_tensor(onesrow_np, "onesrow_const")
    J_dr = nc.inline_tensor(J_np, "j_const")
    I_dr = nc.inline_tensor(I_np, "i_const")

    with tile.TileContext(nc) as tc:
        with tc.tile_pool(name="gpool", bufs=1) as gpool:
            # ---------- persistent ----------
            wp_q = [gpool.tile([128, 12, H], bft, name=f"wpq{q}") for q in range(4)]
            wc_sb = gpool.tile([128, 6, C], bft)
            ts_ck = [gpool.tile([128, 6, CW], bft, name=f"tsck{ck}") for ck in range(NCH)]
            tpJ = [gpool.tile([NE, E], bft, name=f"tpJ{b}") for b in range(B)]
            hpJ = [gpool.tile([NE, 96], bft, name=f"hpJ{b}") for b in range(B)]
            eaT = gpool.tile([128, 4 * B, NE * NH], bft)
            bt_sb = gpool.tile([128, 6], f32)
            bh_sb = gpool.tile([96, 1], f32)
            ones_sb = gpool.tile([128, 1], bft)
            onesrow_sb = gpool.tile([1, 128], f32)
            J_sb = gpool.tile([NE, CW], bft)
            I_sb = gpool.tile([NE, 2 * CW], bft)
            nc.gpsimd.dma_start(out=bt_sb, in_=bt_t[:])
            nc.gpsimd.dma_start(out=bh_sb, in_=bh_c[:])
            nc.gpsimd.dma_start(out=ones_sb, in_=ones_dr[:])
            nc.gpsimd.dma_start(out=onesrow_sb, in_=onesrow_dr[:])
            nc.gpsimd.dma_start(out=J_sb, in_=J_dr[:])
            nc.gpsimd.dma_start(out=I_sb, in_=I_dr[:])

            with (
                tc.tile_pool(name="p1", bufs=1) as p1,
                tc.tile_pool(name="ps1", bufs=1, space="PSUM") as ps1,
            ):
                # ---------- main-phase inputs ----------
                seq_sb = p1.tile([128, 4 * B, H], bft)
                wh2_sb = p1.tile([128, 6, 96], bft)
                wt2_sb = p1.tile([128, 6, E], bft)

                with (
                    tc.tile_pool(name="p0", bufs=1) as p0,
                    tc.tile_pool(name="ps0", bufs=1, space="PSUM") as ps0,
                ):
                    # ---------- early inputs, attention first ----------
                    att_sb = p0.tile([128, 4 * B, NM, NE * NH], bft)
                    for b in range(B):
                        nc.sync.dma_start(
                            out=att_sb[:, 4 * b: 4 * b + 4, :, :],
                            in_=att_T[b].rearrange("(a p) (m x) -> p a m x", p=128, m=NM))
                    ment_sb = p0.tile([96, B, H], bft)
                    nc.scalar.dma_start(out=ment_sb, in_=ment[:])
                    S2_sb = p0.tile([96, NE], bft)
                    nc.scalar.dma_start(out=S2_sb, in_=S2_dr[:])
                    wh1_sb = p0.tile([128, 6, 96], bft)
                    nc.scalar.dma_start(out=wh1_sb, in_=wh1p[:].rearrange("(a p) e -> p a e", p=128))
                    wt1_sb = p0.tile([128, 6, E], bft)
                    nc.scalar.dma_start(out=wt1_sb, in_=wt1[:].rearrange("(a p) e -> p a e", p=128))
                    for b in range(B):
                        nc.sync.dma_start(
                            out=seq_sb[:, 4 * b: 4 * b + 4, :],
                            in_=seq_bf[b].rearrange("(a p) h -> p a h", p=128))
                    nc.scalar.dma_start(out=wh2_sb, in_=wh2p[:].rearrange("(a p) e -> p a e", p=128))
                    nc.scalar.dma_start(out=wt2_sb, in_=wt2[:].rearrange("(a p) e -> p a e", p=128))
                    # phase-2 weights stream in the background, quarter by quarter
                    for q in range(4):
                        nc.sync.dma_start(
                            out=wp_q[q],
                            in_=wp_sl[12 * 128 * q: 12 * 128 * (q + 1), :]
                            .rearrange("(a p) h -> p a h", p=128))
                    nc.scalar.dma_start(out=wc_sb, in_=wc_bf[:].rearrange("(a p) c -> p a c", p=128))

                    # ---------- mention-sum (batch 0 first) ----------
                    ea_a = p0.tile([128, 4 * B, NE * NH], bft)
                    with nc.allow_low_precision("4-term mention-sum in bf16"):
                        for b in range(B):
                            sb = slice(4 * b, 4 * b + 4)
                            nc.vector.tensor_add(out=ea_a[:, sb, :], in0=att_sb[:, sb, 0, :],
                                                 in1=att_sb[:, sb, 1, :])
                            nc.vector.tensor_add(out=eaT[:, sb, :], in0=att_sb[:, sb, 2, :],
                                                 in1=att_sb[:, sb, 3, :])
                            nc.vector.tensor_add(out=eaT[:, sb, :], in0=eaT[:, sb, :],
                                                 in1=ea_a[:, sb, :])

                    # ---------- entity pooling + tpJ/hpJ ----------
                    exp_sb = p0.tile([96, B, H], bft)
                    for b in range(B):
                        nc.scalar.activation(out=exp_sb[:, b, :], in_=ment_sb[:, b, :], func=AF.Exp)
                    eeT = p0.tile([128, 6, B * NE], bft)
                    for ht in range(6):
                        for b in range(B):
                            tr = ps0.tile([128, B * NE], f32, tag="mm48", bufs=2)
                            nc.tensor.matmul(tr[:, 0:NE], lhsT=exp_sb[:, b, 128 * ht: 128 * (ht + 1)],
                                             rhs=S2_sb[:], start=True, stop=True)
                            nc.scalar.activation(out=eeT[:, ht, NE * b: NE * (b + 1)],
                                                 in_=tr[:, 0:NE], func=AF.Ln)
                    for b in range(B):
                        for half in range(2):
                            tp_ps = ps0.tile([NE, 384], f32, tag="tpj", bufs=2)
                            for ht in range(6):
                                nc.tensor.matmul(tp_ps, lhsT=eeT[:, ht, NE * b: NE * (b + 1)],
                                                 rhs=wt1_sb[:, ht, 384 * half: 384 * (half + 1)],
                                                 start=(ht == 0), stop=(ht == 5))
                            nc.scalar.copy(out=tpJ[b][:, 384 * half: 384 * (half + 1)], in_=tp_ps)
                        hp_ps = ps0.tile([NE, 384], f32, tag="tpj", bufs=2)
                        for ht in range(6):
                            nc.tensor.matmul(hp_ps[:, 0:96], lhsT=eeT[:, ht, NE * b: NE * (b + 1)],
                                             rhs=wh1_sb[:, ht, :], start=(ht == 0), stop=(ht == 5))
                        nc.scalar.copy(out=hpJ[b], in_=hp_ps[:, 0:96])
                # ---- p0/ps0 closed: att/ment/exp/wh1/wt1/eeT freed ----

                with (
                    tc.tile_pool(name="p2", bufs=1) as p2,
                    tc.tile_pool(name="ps2", bufs=1, space="PSUM") as ps2,
                ):
                    for ck in range(NCH):
                        b, ih = ck // 2, ck % 2
                        # ---------- pair products (DVE) ----------
                        ht_raw = p1.tile([128, 4, CW], bft, tag="htraw", bufs=2, name=f"htraw{ck}")
                        with nc.allow_low_precision("bf16 pair products; scale-invariant"):
                            for lt in range(4):
                                blt = 4 * b + lt
                                prod = p1.tile([128, 12, NE, NH], bft, tag="prod", bufs=2)
                                in0 = (eaT[:, blt, :].rearrange("p (e h) -> p e h", h=NH)
                                       [:, 12 * ih: 12 * (ih + 1), :]
                                       .unsqueeze(2).broadcast_to([128, 12, NE, NH]))
                                in1 = (eaT[:, blt, :].rearrange("p (e h) -> p e h", h=NH)
                                       .unsqueeze(1).broadcast_to([128, 12, NE, NH]))
                                nc.vector.tensor_mul(out=prod, in0=in0, in1=in1)
                                nc.vector.tensor_reduce(
                                    out=ht_raw[:, lt, :],
                                    in_=prod[:].rearrange("p a b h -> p (a b) h"),
                                    axis=X, op=ALU.add)
                            nc.vector.tensor_scalar_max(out=ht_raw[:], in0=ht_raw[:], scalar1=0.0)

                        # ---------- l-normalization ----------
                        sum_ps = ps2.tile([128, CW], f32, tag="big", bufs=2)
                        for lt in range(4):
                            nc.tensor.matmul(sum_ps[0:1, :], lhsT=ones_sb[:], rhs=ht_raw[:, lt, :],
                                             start=(lt == 0), stop=(lt == 3))
                        den_s = p1.tile([1, CW], f32, tag="dens", bufs=2)
                        nc.vector.tensor_scalar_add(out=den_s, in0=sum_ps[0:1, :], scalar1=1e-10)
                        den_r = p1.tile([1, CW], f32, tag="denr", bufs=2)
                        nc.vector.reciprocal_approx_fast(out=den_r, in_=den_s[:])
                        rep_ps = ps2.tile([128, CW], f32, tag="big", bufs=2)
                        nc.tensor.matmul(rep_ps, lhsT=onesrow_sb[:], rhs=den_r[:], start=True, stop=True)
                        rrep = p1.tile([128, CW], bft, tag="rrep", bufs=2)
                        nc.vector.tensor_copy(out=rrep, in_=rep_ps)
                        htn = p1.tile([128, 4, CW], bft, tag="htn", bufs=2, name=f"htn{ck}")
                        for lt in range(4):
                            nc.vector.tensor_mul(out=htn[:, lt, :], in0=ht_raw[:, lt, :], in1=rrep)

                        # ---------- rsT ----------
                        rsT = p1.tile([128, 6, CW], bft, tag="rst", bufs=2, name=f"rst{ck}")
                        for ht in range(6):
                            rp = ps2.tile([128, CW], f32, tag="big", bufs=2)
                            for lt in range(4):
                                nc.tensor.matmul(rp, lhsT=seq_sb[:, 4 * b + lt, 128 * ht: 128 * (ht + 1)],
                                                 rhs=htn[:, lt, :], start=(lt == 0), stop=(lt == 3))
                            nc.scalar.copy(out=rsT[:, ht, :], in_=rp)

                        # ---------- hs extractor -> hs_dr ----------
                        hp3 = ps2.tile([128, CW], f32, tag="big", bufs=2)
                        for ht in range(6):
                            nc.tensor.matmul(hp3[0:96, :], lhsT=wh2_sb[:, ht, :], rhs=rsT[:, ht, :],
                                             start=(ht == 0), stop=False)
                        nc.tensor.matmul(hp3[0:96, :], lhsT=hpJ[b][:],
                                         rhs=I_sb[:, CW * ih: CW * (ih + 1)], start=False, stop=True)
                        hs_t = p1.tile([96, CW], bft, tag="hst", bufs=2)
                        nc.scalar.activation(out=hs_t, in_=hp3[0:96, :], func=AF.Tanh,
                                             bias=bh_sb[:, 0:1])
                        nc.sync.dma_start(out=hs_dr[:, CW * ck: CW * (ck + 1)], in_=hs_t[:])

                        # ---------- ts extractor ----------
                        for Et in range(6):
                            ep = ps2.tile([128, CW], f32, tag="big", bufs=2)
                            for ht in range(6):
                                nc.tensor.matmul(ep, lhsT=wt2_sb[:, ht, 128 * Et: 128 * (Et + 1)],
                                                 rhs=rsT[:, ht, :], start=(ht == 0), stop=False)
                            nc.tensor.matmul(ep, lhsT=tpJ[b][:, 128 * Et: 128 * (Et + 1)],
                                             rhs=J_sb[:], start=False, stop=True)
                            nc.scalar.activation(out=ts_ck[ck][:, Et, :],
                                                 in_=ep, func=AF.Tanh, bias=bt_sb[:, Et: Et + 1])

                        # ---------- phase 2 for this chunk ----------
                        fps = []
                        for h in range(6):
                            fps.append(ps2.tile([128, CW], f32, tag=f"feat{h}", bufs=1,
                                                name=f"fps{h}"))
                        # 3-wide bl groups: one DVE multiply + 18 matmuls per
                        # group -> 3x fewer PE<-DVE sync points
                        for grp in range(NT2 // 3):
                            s_l, ah = grp // 2, grp % 2
                            b1r = p2.tile([128, 3, CW], bft, tag="b1r", bufs=6)
                            for j in range(3):
                                a = 3 * ah + j
                                r0 = (2 * a) * SL + s_l
                                r1 = (2 * a + 1) * SL + s_l
                                eng = nc.sync if j % 2 == 0 else nc.scalar
                                eng.dma_start(
                                    out=b1r[0:64, j, :],
                                    in_=bass.AP(tensor=hs_dr, offset=r0 * NP + CW * ck,
                                                ap=[[0, 64], [1, CW]]))
                                eng.dma_start(
                                    out=b1r[64:128, j, :],
                                    in_=bass.AP(tensor=hs_dr, offset=r1 * NP + CW * ck,
                                                ap=[[0, 64], [1, CW]]))
                            bl = p2.tile([128, 3, CW], bft, tag="bl", bufs=4)
                            mul_eng = nc.gpsimd if grp % 3 == 2 else nc.vector
                            mul_eng.tensor_mul(out=bl, in0=b1r,
                                               in1=ts_ck[ck][:, 3 * ah: 3 * ah + 3, :])
                            for j in range(3):
                                kt2 = s_l * 6 + 3 * ah + j
                                for h in range(6):
                                    nc.tensor.matmul(fps[h], lhsT=wp_q[kt2 // 12][:, kt2 % 12, 128 * h: 128 * (h + 1)],
                                                     rhs=bl[:, j, :], start=(kt2 == 0), stop=(kt2 == NT2 - 1))
                        lgp = ps2.tile([128, CW], f32, tag="big", bufs=2)
                        for h in range(6):
                            fT = p2.tile([128, CW], bft, tag="fT", bufs=3)
                            if h % 2 == 0:
                                nc.scalar.copy(out=fT, in_=fps[h])
                            else:
                                nc.vector.tensor_copy(out=fT, in_=fps[h])
                            nc.tensor.matmul(lgp[0:C, :], lhsT=wc_sb[:, h, :], rhs=fT,
                                             start=(h == 0), stop=(h == 5))
                        lgo = p2.tile([C, CW], f32, tag="lgo", bufs=2)
                        nc.vector.tensor_copy(out=lgo, in_=lgp[0:C, :])
                        nc.sync.dma_start(out=out_lgT[:, CW * ck: CW * (ck + 1)], in_=lgo[:])

    if not nc.is_finalized():
        nc.finalize()
    return nc


_NC_CACHE = None


def _host_finish(partials, bc):
    lgT = np.sum([np.asarray(p, dtype=np.float32) for p in partials], axis=0)
    logits = np.ascontiguousarray(lgT.T) + bc[None, :]
    mask = np.ones((NP, 1), np.float32)
    for b in range(B):
        for i in range(NE):
            mask[b * PPB + i * NE + i, 0] = 0.0
    return (logits * mask).astype(np.float32)


def kernel(**inputs):
    global _NC_CACHE
    from concourse.bass_utils import run_bass_kernel_spmd

    if _NC_CACHE is None:
        _NC_CACHE = build_bass()
    in_maps = _host_prep(inputs)
    res = run_bass_kernel_spmd(_NC_CACHE, in_maps, core_ids=list(range(NCORE)))
    kernel.last_results = res
    bc = np.asarray(inputs["bc"], dtype=np.float32)
    return _host_finish([res.results[c]["out_lgT"] for c in range(NCORE)], bc)


# revision 6
# speedup vs baseline: 1.1095x; 1.1095x over previous
"""Trainium2 Bass kernel v4 for nn_DocREModel_Triangle — collective-free,
phase-interleaved.

v4 over v3 (trace-driven):
  - full per-chunk pipeline: for each 288-pair chunk, products (DVE) ->
    normalization -> rsT -> extractors -> phase-2 feature+classifier are
    emitted together, so the PE works on chunk k's matmuls while the DVE
    computes chunk k+1's pair products (v3 ran the ~90us DVE product
    phase with the PE mostly idle, then a PE-bound phase 2).
  - gpsimd no longer issues DMAs (software-DGE dispatch was ~620ns per
    dma_start, 60us total) and no longer runs the big product muls
    (7.1us each vs 1.9us on DVE). It only takes 1/3 of the small
    phase-2 bl muls.
  - phase-2 chunk = 288 pairs (aligned with phase 1); b1 broadcast rows
    loaded just-in-time per tile (2 stride-0 DMAs, bufs=16, issued
    alternately on the two hardware DGE queues).
  - ts stored per-chunk (4 tiles) to avoid false tile-level WAR hazards
    between chunk k's phase-2 reads and chunk k+1's extractor writes.
  - wp loaded as 4 quarter-tiles so early phase-2 matmuls of chunk 0
    don't wait for the full 9.4 MB weight load.
  - PSUM: 6 feature accumulators + one shared [128,288] f32 tag (rp/ep/
    sums/replicate/classifier) with 2 bufs = 8 banks exactly.
"""

import numpy as np
import ml_dtypes

bf16 = ml_dtypes.bfloat16

B, L, H, NH = 2, 512, 768, 12
NE, NM = 24, 4
E, BS, C = 768, 64, 97
K = E // BS
NCORE = 8
SL = 64 // NCORE                 # 8 s-values per core
NP = B * NE * NE                 # 1152 pairs
PPB = NE * NE                    # 576 pairs per batch
CW = 288                         # pair chunk (12 i x 24 j)
NCH = NP // CW                   # 4
KST = K * SL * BS                # 6144 contraction rows per core
NT2 = KST // 128                 # 48 tiles, kt2 = s*6 + a (a = k//2)


def _host_prep(inputs):
    seq = np.ascontiguousarray(inputs["sequence_output"], dtype=np.float32)
    att = np.ascontiguousarray(inputs["attention"], dtype=np.float32)
    Wh = np.asarray(inputs["Wh"], dtype=np.float32)
    Wt = np.asarray(inputs["Wt"], dtype=np.float32)
    Wp = np.asarray(inputs["Wp"], dtype=np.float32)
    Wc = np.asarray(inputs["Wc"], dtype=np.float32)
    bh = np.asarray(inputs["bh"], dtype=np.float32)
    bt = np.asarray(inputs["bt"], dtype=np.float32)
    mpos = np.asarray(inputs["mention_pos"]).astype(np.int64)

    seq_bf = np.ascontiguousarray(seq.astype(bf16))
    wt1 = np.ascontiguousarray(Wt[:H].astype(bf16))
    wt2 = np.ascontiguousarray(Wt[H:].astype(bf16))
    bt_t = np.ascontiguousarray(bt.reshape(6, 128).T.astype(np.float32))
    wc_bf = np.ascontiguousarray(Wc.astype(bf16))
    wp4 = Wp.reshape(K, 64, BS, H)

    # mention rows (m, e): row = m*24 + e  -> [96, 2, 768] bf16
    ment = np.ascontiguousarray(
        np.stack([seq[b][mpos[b].T.reshape(-1)] for b in range(B)], 1)
        .astype(bf16))
    # attention gather, transposed, m-major: [b, l, (m e h)] bf16
    attT = np.zeros((B, L, NM * NE * NH), bf16)
    for b in range(B):
        G = att[b][:, mpos[b].reshape(-1), :]                 # [12, 96, 512]
        X = G.reshape(NH, NE, NM, L).transpose(3, 2, 1, 0)    # [512, 4m, 24e, 12h]
        attT[b] = X.reshape(L, -1).astype(bf16)
    attT = np.ascontiguousarray(attT)

    in_maps = []
    for c in range(NCORE):
        PERM_c = np.array([k * 64 + SL * c + sl for k in range(K) for sl in range(SL)])
        wh1p = np.ascontiguousarray(Wh[:H][:, PERM_c].astype(bf16))   # [768, 96]
        wh2p = np.ascontiguousarray(Wh[H:][:, PERM_c].astype(bf16))
        bh_c = np.ascontiguousarray(bh[PERM_c].reshape(96, 1).astype(np.float32))
        wp_c = np.ascontiguousarray(
            wp4[:, SL * c: SL * (c + 1)].transpose(1, 0, 2, 3)
            .reshape(KST, H).astype(bf16))
        in_maps.append({
            "seq_bf": seq_bf,
            "att_T": attT,
            "ment": ment,
            "wh1p": wh1p,
            "wh2p": wh2p,
            "wt1": wt1,
            "wt2": wt2,
            "bh_c": bh_c,
            "bt_t": bt_t,
            "wp_sl": wp_c,
            "wc_bf": wc_bf,
        })
    return in_maps


def _build_consts():
    S2 = np.zeros((96, NE), dtype=bf16)
    for m in range(NM):
        for e in range(NE):
            S2[m * NE + e, e] = 1.0
    ones_bf = np.ones((128, 1), dtype=bf16)
    ones_row = np.ones((1, 128), dtype=np.float32)
    Jind = np.zeros((NE, CW), dtype=bf16)
    for il in range(12):
        for j in range(NE):
            Jind[j, il * NE + j] = 1.0
    Iind = np.zeros((NE, 2 * CW), dtype=bf16)
    for ih in range(2):
        for il in range(12):
            for j in range(NE):
                Iind[ih * 12 + il, ih * CW + il * NE + j] = 1.0
    return S2, ones_bf, ones_row, Jind, Iind


def build_bass():
    import concourse.bass as bass
    import concourse.mybir as mybir
    import concourse.tile as tile
    from concourse.bacc import Bacc

    f32 = mybir.dt.float32
    bft = mybir.dt.bfloat16
    AF = mybir.ActivationFunctionType
    ALU = mybir.AluOpType
    X = mybir.AxisListType.X

    nc = Bacc("TRN2", num_devices=NCORE)

    seq_bf = nc.dram_tensor("seq_bf", [B, L, H], bft, kind="ExternalInput")
    att_T = nc.dram_tensor("att_T", [B, L, NM * NE * NH], bft, kind="ExternalInput")
    ment = nc.dram_tensor("ment", [96, B, H], bft, kind="ExternalInput")
    wh1p = nc.dram_tensor("wh1p", [H, 96], bft, kind="ExternalInput")
    wh2p = nc.dram_tensor("wh2p", [H, 96], bft, kind="ExternalInput")
    wt1 = nc.dram_tensor("wt1", [H, E], bft, kind="ExternalInput")
    wt2 = nc.dram_tensor("wt2", [H, E], bft, kind="ExternalInput")
    bh_c = nc.dram_tensor("bh_c", [96, 1], f32, kind="ExternalInput")
    bt_t = nc.dram_tensor("bt_t", [128, 6], f32, kind="ExternalInput")
    wp_sl = nc.dram_tensor("wp_sl", [KST, H], bft, kind="ExternalInput")
    wc_bf = nc.dram_tensor("wc_bf", [H, C], bft, kind="ExternalInput")
    out_lgT = nc.dram_tensor("out_lgT", [C, NP], f32, kind="ExternalOutput")

    hs_dr = nc.dram_tensor("hs_dr", [96, NP], bft)

    S2_np, ones_np, onesrow_np, J_np, I_np = _build_consts()
    S2_dr = nc.inline_tensor(S2_np, "s2_const")
    ones_dr = nc.inline_tensor(ones_np, "ones_const")
    onesrow_dr = nc.inline_tensor(onesrow_np, "onesrow_const")
    J_dr = nc.inline_tensor(J_np, "j_const")
    I_dr = nc.inline_tensor(I_np, "i_const")

    with tile.TileContext(nc) as tc:
        with tc.tile_pool(name="gpool", bufs=1) as gpool:
            # ---------- persistent ----------
            wp_q = [gpool.tile([128, 12, H], bft, name=f"wpq{q}") for q in range(4)]
            wc_sb = gpool.tile([128, 6, C], bft)
            ts_ck = [gpool.tile([128, 6, CW], bft, name=f"tsck{ck}") for ck in range(NCH)]
            tpJ = [gpool.tile([NE, E], bft, name=f"tpJ{b}") for b in range(B)]
            hpJ = [gpool.tile([NE, 96], bft, name=f"hpJ{b}") for b in range(B)]
            eaT = gpool.tile([128, 4 * B, NE * NH], bft)
            bt_sb = gpool.tile([128, 6], f32)
            bh_sb = gpool.tile([96, 1], f32)
            ones_sb = gpool.tile([128, 1], bft)
            onesrow_sb = gpool.tile([1, 128], f32)
            J_sb = gpool.tile([NE, CW], bft)
            I_sb = gpool.tile([NE, 2 * CW], bft)
            nc.gpsimd.dma_start(out=bt_sb, in_=bt_t[:])
            nc.gpsimd.dma_start(out=bh_sb, in_=bh_c[:])
            nc.gpsimd.dma_start(out=ones_sb, in_=ones_dr[:])
            nc.gpsimd.dma_start(out=onesrow_sb, in_=onesrow_dr[:])
            nc.gpsimd.dma_start(out=J_sb, in_=J_dr[:])
            nc.gpsimd.dma_start(out=I_sb, in_=I_dr[:])

            with (
                tc.tile_pool(name="p1", bufs=1) as p1,
                tc.tile_pool(name="ps1", bufs=1, space="PSUM") as ps1,
            ):
                # ---------- main-phase inputs ----------
                seq_sb = p1.tile([128, 4 * B, H], bft)
                wh2_sb = p1.tile([128, 6, 96], bft)
                wt2_sb = p1.tile([128, 6, E], bft)

                with (
                    tc.tile_pool(name="p0", bufs=1) as p0,
                    tc.tile_pool(name="ps0", bufs=1, space="PSUM") as ps0,
                ):
                    # ---------- early inputs, attention first ----------
                    att_sb = p0.tile([128, 4 * B, NM, NE * NH], bft)
                    for b in range(B):
                        nc.sync.dma_start(
                            out=att_sb[:, 4 * b: 4 * b + 4, :, :],
                            in_=att_T[b].rearrange("(a p) (m x) -> p a m x", p=128, m=NM))
                    ment_sb = p0.tile([96, B, H], bft)
                    nc.scalar.dma_start(out=ment_sb, in_=ment[:])
                    S2_sb = p0.tile([96, NE], bft)
                    nc.scalar.dma_start(out=S2_sb, in_=S2_dr[:])
                    wh1_sb = p0.tile([128, 6, 96], bft)
                    nc.scalar.dma_start(out=wh1_sb, in_=wh1p[:].rearrange("(a p) e -> p a e", p=128))
                    wt1_sb = p0.tile([128, 6, E], bft)
                    nc.scalar.dma_start(out=wt1_sb, in_=wt1[:].rearrange("(a p) e -> p a e", p=128))
                    for b in range(B):
                        nc.sync.dma_start(
                            out=seq_sb[:, 4 * b: 4 * b + 4, :],
                            in_=seq_bf[b].rearrange("(a p) h -> p a h", p=128))
                    nc.scalar.dma_start(out=wh2_sb, in_=wh2p[:].rearrange("(a p) e -> p a e", p=128))
                    nc.scalar.dma_start(out=wt2_sb, in_=wt2[:].rearrange("(a p) e -> p a e", p=128))
                    # phase-2 weights stream in the background, quarter by quarter
                    for q in range(4):
                        nc.sync.dma_start(
                            out=wp_q[q],
                            in_=wp_sl[12 * 128 * q: 12 * 128 * (q + 1), :]
                            .rearrange("(a p) h -> p a h", p=128))
                    nc.scalar.dma_start(out=wc_sb, in_=wc_bf[:].rearrange("(a p) c -> p a c", p=128))

                    # ---------- mention-sum (batch 0 first) ----------
                    ea_a = p0.tile([128, 4 * B, NE * NH], bft)
                    with nc.allow_low_precision("4-term mention-sum in bf16"):
                        for b in range(B):
                            sb = slice(4 * b, 4 * b + 4)
                            nc.vector.tensor_add(out=ea_a[:, sb, :], in0=att_sb[:, sb, 0, :],
                                                 in1=att_sb[:, sb, 1, :])
                            nc.vector.tensor_add(out=eaT[:, sb, :], in0=att_sb[:, sb, 2, :],
                                                 in1=att_sb[:, sb, 3, :])
                            nc.vector.tensor_add(out=eaT[:, sb, :], in0=eaT[:, sb, :],
                                                 in1=ea_a[:, sb, :])

                    # ---------- entity pooling + tpJ/hpJ ----------
                    exp_sb = p0.tile([96, B, H], bft)
                    for b in range(B):
                        nc.scalar.activation(out=exp_sb[:, b, :], in_=ment_sb[:, b, :], func=AF.Exp)
                    eeT = p0.tile([128, 6, B * NE], bft)
                    for ht in range(6):
                        for b in range(B):
                            tr = ps0.tile([128, B * NE], f32, tag="mm48", bufs=2)
                            nc.tensor.matmul(tr[:, 0:NE], lhsT=exp_sb[:, b, 128 * ht: 128 * (ht + 1)],
                                             rhs=S2_sb[:], start=True, stop=True)
                            nc.scalar.activation(out=eeT[:, ht, NE * b: NE * (b + 1)],
                                                 in_=tr[:, 0:NE], func=AF.Ln)
                    for b in range(B):
                        for half in range(2):
                            tp_ps = ps0.tile([NE, 384], f32, tag="tpj", bufs=2)
                            for ht in range(6):
                                nc.tensor.matmul(tp_ps, lhsT=eeT[:, ht, NE * b: NE * (b + 1)],
                                                 rhs=wt1_sb[:, ht, 384 * half: 384 * (half + 1)],
                                                 start=(ht == 0), stop=(ht == 5))
                            nc.scalar.copy(out=tpJ[b][:, 384 * half: 384 * (half + 1)], in_=tp_ps)
                        hp_ps = ps0.tile([NE, 384], f32, tag="tpj", bufs=2)
                        for ht in range(6):
                            nc.tensor.matmul(hp_ps[:, 0:96], lhsT=eeT[:, ht, NE * b: NE * (b + 1)],
                                             rhs=wh1_sb[:, ht, :], start=(ht == 0), stop=(ht == 5))
                        nc.scalar.copy(out=hpJ[b], in_=hp_ps[:, 0:96])
                # ---- p0/ps0 closed: att/ment/exp/wh1/wt1/eeT freed ----

                with (
                    tc.tile_pool(name="p2", bufs=1) as p2,
                    tc.tile_pool(name="ps2", bufs=1, space="PSUM") as ps2,
                ):
                    for ck in range(NCH):
                        b, ih = ck // 2, ck % 2
                        # ---------- pair products (DVE) ----------
                        ht_raw = p1.tile([128, 4, CW], bft, tag="htraw", bufs=2, name=f"htraw{ck}")
                        with nc.allow_low_precision("bf16 pair products; scale-invariant"):
                            for lt in range(4):
                                blt = 4 * b + lt
                                prod = p1.tile([128, 12, NE, NH], bft, tag="prod", bufs=2)
                                in0 = (eaT[:, blt, :].rearrange("p (e h) -> p e h", h=NH)
                                       [:, 12 * ih: 12 * (ih + 1), :]
                                       .unsqueeze(2).broadcast_to([128, 12, NE, NH]))
                                in1 = (eaT[:, blt, :].rearrange("p (e h) -> p e h", h=NH)
                                       .unsqueeze(1).broadcast_to([128, 12, NE, NH]))
                                nc.vector.tensor_mul(out=prod, in0=in0, in1=in1)
                                nc.vector.tensor_reduce(
                                    out=ht_raw[:, lt, :],
                                    in_=prod[:].rearrange("p a b h -> p (a b) h"),
                                    axis=X, op=ALU.add)
                            nc.vector.tensor_scalar_max(out=ht_raw[:], in0=ht_raw[:], scalar1=0.0)

                        # ---------- l-normalization ----------
                        sum_ps = ps2.tile([128, CW], f32, tag="big", bufs=2)
                        for lt in range(4):
                            nc.tensor.matmul(sum_ps[0:1, :], lhsT=ones_sb[:], rhs=ht_raw[:, lt, :],
                                             start=(lt == 0), stop=(lt == 3))
                        den_s = p1.tile([1, CW], f32, tag="dens", bufs=2)
                        nc.vector.tensor_scalar_add(out=den_s, in0=sum_ps[0:1, :], scalar1=1e-10)
                        den_r = p1.tile([1, CW], f32, tag="denr", bufs=2)
                        nc.vector.reciprocal_approx_fast(out=den_r, in_=den_s[:])
                        rep_ps = ps2.tile([128, CW], f32, tag="big", bufs=2)
                        nc.tensor.matmul(rep_ps, lhsT=onesrow_sb[:], rhs=den_r[:], start=True, stop=True)
                        rrep = p1.tile([128, CW], bft, tag="rrep", bufs=2)
                        nc.vector.tensor_copy(out=rrep, in_=rep_ps)
                        htn = p1.tile([128, 4, CW], bft, tag="htn", bufs=2, name=f"htn{ck}")
                        for lt in range(4):
                            nc.vector.tensor_mul(out=htn[:, lt, :], in0=ht_raw[:, lt, :], in1=rrep)

                        # ---------- rsT ----------
                        rsT = p1.tile([128, 6, CW], bft, tag="rst", bufs=2, name=f"rst{ck}")
                        for ht in range(6):
                            rp = ps2.tile([128, CW], f32, tag="big", bufs=2)
                            for lt in range(4):
                                nc.tensor.matmul(rp, lhsT=seq_sb[:, 4 * b + lt, 128 * ht: 128 * (ht + 1)],
                                                 rhs=htn[:, lt, :], start=(lt == 0), stop=(lt == 3))
                            nc.scalar.copy(out=rsT[:, ht, :], in_=rp)

                        # ---------- hs extractor -> hs_dr ----------
                        hp3 = ps2.tile([128, CW], f32, tag="big", bufs=2)
                        for ht in range(6):
                            nc.tensor.matmul(hp3[0:96, :], lhsT=wh2_sb[:, ht, :], rhs=rsT[:, ht, :],
                                             start=(ht == 0), stop=False)
                        nc.tensor.matmul(hp3[0:96, :], lhsT=hpJ[b][:],
                                         rhs=I_sb[:, CW * ih: CW * (ih + 1)], start=False, stop=True)
                        hs_t = p1.tile([96, CW], bft, tag="hst", bufs=2)
                        nc.scalar.activation(out=hs_t, in_=hp3[0:96, :], func=AF.Tanh,
                                             bias=bh_sb[:, 0:1])
                        nc.sync.dma_start(out=hs_dr[:, CW * ck: CW * (ck + 1)], in_=hs_t[:])

                        # ---------- ts extractor ----------
                        for Et in range(6):
                            ep = ps2.tile([128, CW], f32, tag="big", bufs=2)
                            for ht in range(6):
                                nc.tensor.matmul(ep, lhsT=wt2_sb[:, ht, 128 * Et: 128 * (Et + 1)],
                                                 rhs=rsT[:, ht, :], start=(ht == 0), stop=False)
                            nc.tensor.matmul(ep, lhsT=tpJ[b][:, 128 * Et: 128 * (Et + 1)],
                                             rhs=J_sb[:], start=False, stop=True)
                            nc.scalar.activation(out=ts_ck[ck][:, Et, :],
                                                 in_=ep, func=AF.Tanh, bias=bt_sb[:, Et: Et + 1])

                        # ---------- phase 2 for this chunk ----------
                        fps = []
                        for h in range(6):
                            fps.append(ps2.tile([128, CW], f32, tag=f"feat{h}", bufs=1,
                                                name=f"fps{h}"))
                        for kt2 in range(NT2):
                            s_l, a = kt2 // 6, kt2 % 6
                            r0 = (2 * a) * SL + s_l
                            r1 = (2 * a + 1) * SL + s_l
                            b1r = p2.tile([128, CW], bft, tag="b1r", bufs=16)
                            eng = nc.sync if kt2 % 2 == 0 else nc.scalar
                            eng.dma_start(
                                out=b1r[0:64, :],
                                in_=bass.AP(tensor=hs_dr, offset=r0 * NP + CW * ck,
                                            ap=[[0, 64], [1, CW]]))
                            eng.dma_start(
                                out=b1r[64:128, :],
                                in_=bass.AP(tensor=hs_dr, offset=r1 * NP + CW * ck,
                                            ap=[[0, 64], [1, CW]]))
                            bl = p2.tile([128, CW], bft, tag="bl", bufs=8)
                            mul_eng = nc.gpsimd if kt2 % 3 == 2 else nc.vector
                            mul_eng.tensor_mul(out=bl, in0=b1r, in1=ts_ck[ck][:, a, :])
                            for h in range(6):
                                nc.tensor.matmul(fps[h], lhsT=wp_q[kt2 // 12][:, kt2 % 12, 128 * h: 128 * (h + 1)],
                                                 rhs=bl, start=(kt2 == 0), stop=(kt2 == NT2 - 1))
                        lgp = ps2.tile([128, CW], f32, tag="big", bufs=2)
                        for h in range(6):
                            fT = p2.tile([128, CW], bft, tag="fT", bufs=3)
                            if h % 2 == 0:
                                nc.scalar.copy(out=fT, in_=fps[h])
                            else:
                                nc.vector.tensor_copy(out=fT, in_=fps[h])
                            nc.tensor.matmul(lgp[0:C, :], lhsT=wc_sb[:, h, :], rhs=fT,
                                             start=(h == 0), stop=(h == 5))
                        lgo = p2.tile([C, CW], f32, tag="lgo", bufs=2)
                        nc.vector.tensor_copy(out=lgo, in_=lgp[0:C, :])
                        nc.sync.dma_start(out=out_lgT[:, CW * ck: CW * (ck + 1)], in_=lgo[:])

    if not nc.is_finalized():
        nc.finalize()
    return nc


_NC_CACHE = None


def _host_finish(partials, bc):
    lgT = np.sum([np.asarray(p, dtype=np.float32) for p in partials], axis=0)
    logits = np.ascontiguousarray(lgT.T) + bc[None, :]
    mask = np.ones((NP, 1), np.float32)
    for b in range(B):
        for i in range(NE):
            mask[b * PPB + i * NE + i, 0] = 0.0
    return (logits * mask).astype(np.float32)


def kernel(**inputs):
    global _NC_CACHE
    from concourse.bass_utils import run_bass_kernel_spmd

    if _NC_CACHE is None:
        _NC_CACHE = build_bass()
    in_maps = _host_prep(inputs)
    res = run_bass_kernel_spmd(_NC_CACHE, in_maps, core_ids=list(range(NCORE)))
    kernel.last_results = res
    bc = np.asarray(inputs["bc"], dtype=np.float32)
    return _host_finish([res.results[c]["out_lgT"] for c in range(NCORE)], bc)
